# revision 4
# baseline (speedup 1.0000x reference)
"""LinearAttention kernel for one TRN2 chip (8 NeuronCores), Bass/Tile.

Math (per batch b):
  qkv = x @ w_qkv.T ; q,k,v split, per-head [n, 64]
  k_s = softmax(k, axis=-1)              (over dh, per token/head)
  context_h = k_s^T @ v                  [64, 64]
  out_h = q_h @ context_h ; y = out @ w_out.T + b
Restructured as:
  CT_h = (v/s)-weighted partial:  CT[e,d] = sum_n v[n,e]/s[n,h] * exp(k[n,d])
  G_h  = context_h @ w_out_h^T   -> G [inner=512, 1024] block rows
  y    = q @ G + b               (single K=512 matmul)

Sharding: 8 shards = (batch, half-sequence); each core computes its
2048 tokens end-to-end; only the tiny per-batch context (128 KiB) is
all-reduced between the two cores sharing a batch.

All-bf16 compute. fp8 was MEASURED numerically unsafe everywhere:
weight quantization is systematic across tokens (4e-2), and per-token
activation noise does NOT average out relative to the context, whose
random part also grows as sqrt(N) (1.8e-2 for either x8-kv or fp8-CT,
vs the 2e-2 gate and bf16's 5.4e-3).

Schedule (v6).  Measured invariants that shaped it: DMA rings start
~9-14us after kernel start and sustain ~145 GB/s each (sync + gpsimd;
scalar can also issue DMAs); the PE p-state needs ~3us of continuous
busy to reach 2.4 GHz and idle gaps can re-throttle it (HAM k=4 =
half clock); the FIRST collective pays ~11.5us one-time CC-stream
init; collective_compute BLOCKS the issuing gpsimd queue until the
transfer completes; exec time tracks the LAGGARD core (cross-core DMA
contention skews cores by 5-15us), so collectives must never gate PE
work that could run earlier.
  A. 14 warmup matmuls keep the PE busy/ramping through the DMA
     spin-up window.
  B. q projection for tokens 0-1023, ck-outer / i-inner into 4
     accumulating PSUM banks, so consumption matches DMA arrival
     chunk-for-chunk (ring1 wq, ring2 x) while wkv-k repays the window.
  C. k-only projection tiles 0-3 (needs just the k-half of wkv).
  D. combined k+v tiles 4-15, then v-only coda tiles 0-3.  Per v-tile:
     8 v matmuls then 4 pair-packed CT matmuls (2 heads per 128-col
     matmul; off-diagonal 64x64 blocks computed-and-ignored; diagonals
     land in the packed layout the collective wants; `start` only on
     the first pair -- start marks the whole PSUM bank pending-zero).
     CT for v-position j issues after the v matmuls of position j+1
     (one-tile lag) so the DVE vsc chain never head-of-line blocks the
     PE FIFO.  CT stage 0 = first 4 v-tiles: it rendezvouses the pair
     EARLY and completes before stage 1's input exists, so stage 1's
     trigger is not serialized behind it on the gpsimd queue.
  E. final CT + stage-1 collective launch IMMEDIATELY after the coda,
     then the q projection for tokens 1024-2047 absorbs the partner
     skew + transfer + rank-sum chain.
  F. G = blockdiag(context) @ w_out^T (g_s drains on ACT and DVE in
     parallel), then y = q @ G + b; y drains on three DMA rings with
     the last 6 tiles split in halves across ring pairs.
"""

import contextlib
import ctypes
import os
import sys
import types

import numpy as np
import ml_dtypes

# ---------------------------------------------------------------------------
# Compat shim 1: the walrus in this image supports only ONE semaphore wait
# per instruction; split multi-wait instructions into prefix NoOps.
# ---------------------------------------------------------------------------
_MAX_WAITS = 1


def _legalize_bir(bir_bytes: bytes) -> bytes:
    import orjson

    bir = orjson.loads(bir_bytes)
    changed = False
    for fn in bir.get("functions", []):
        for blk in fn.get("blocks", []):
            new_insts = []
            for ins in blk.get("instructions", []):
                si = ins.get("sync_info") or {}
                waits = si.get("on_wait") or []
                if len(waits) > _MAX_WAITS:
                    changed = True
                    extra, keep = waits[:-_MAX_WAITS], waits[-_MAX_WAITS:]
                    for i in range(0, len(extra), _MAX_WAITS):
                        new_insts.append(
                            {
                                "name": f"{ins['name']}-ws{i}",
                                "opcode": "NoOp",
                                "engine": ins["engine"],
                                "ins": [],
                                "outs": [],
                                "sync_info": {
                                    "on_update": [],
                                    "on_wait": extra[i : i + _MAX_WAITS],
                                },
                            }
                        )
                    si["on_wait"] = keep
                new_insts.append(ins)
            blk["instructions"] = new_insts
    if not changed:
        return bir_bytes
    return orjson.dumps(bir)


_compile_patched = False


def _install_compile_patch():
    global _compile_patched
    if _compile_patched:
        return
    import concourse.bass2jax as bass2jax

    orig = bass2jax.compile_bir_kernel

    def compile_bir_kernel_legalized(bir_json, tmpdir, neff_name="file.neff"):
        return orig(_legalize_bir(bytes(bir_json)), tmpdir, neff_name=neff_name)

    bass2jax.compile_bir_kernel = compile_bir_kernel_legalized
    _compile_patched = True


# ---------------------------------------------------------------------------
# Compat shim 2: NTFF profiling hook (only needed when BASS_TRACE is set).
# ---------------------------------------------------------------------------
def _install_ntff_hook():
    import antenv

    if "antenv.axon_hooks" in sys.modules:
        return
    so_path = "/opt/axon/libaxon_pjrt.so"

    def _mk(so_path):
        try:
            lib = ctypes.CDLL(so_path)
        except OSError:
            return None
        if not hasattr(lib, "axon_start_nrt_profile"):
            return None
        lib.axon_start_nrt_profile.argtypes = [
            ctypes.POINTER(ctypes.c_int64),
            ctypes.c_size_t,
        ]
        lib.axon_start_nrt_profile.restype = ctypes.c_int64
        lib.axon_stop_nrt_profile.argtypes = [ctypes.c_char_p]
        lib.axon_stop_nrt_profile.restype = ctypes.c_int64

        @contextlib.contextmanager
        def _hook(output_dir, device_ids):
            import jax

            jax.devices()
            if device_ids:
                ids = (ctypes.c_int64 * len(device_ids))(*device_ids)
                rc = lib.axon_start_nrt_profile(ids, len(device_ids))
            else:
                rc = lib.axon_start_nrt_profile(None, 0)
            if rc != 0:
                raise RuntimeError(f"axon_start_nrt_profile rc={rc}")
            try:
                yield
            finally:
                n = lib.axon_stop_nrt_profile(str(output_dir).encode())
                if n < 0:
                    raise RuntimeError(f"axon_stop_nrt_profile rc={n}")

        return _hook

    hook = _mk(so_path)
    mod = types.ModuleType("antenv.axon_hooks")
    mod.get_axon_ntff_profile_hook = lambda: hook
    mod.set_axon_ntff_profile_hook = lambda h: None
    sys.modules["antenv.axon_hooks"] = mod
    antenv.axon_hooks = mod


# ---------------------------------------------------------------------------
# Kernel
# ---------------------------------------------------------------------------
B, SEQ, D = 4, 4096, 1024
HEADS, DH = 8, 64
INNER = HEADS * DH  # 512
NCORES = 8
NTOK = B * SEQ // NCORES  # 2048 tokens per core
NT = NTOK // 128  # 16
CK = D // 128  # 8 contraction chunks for the qkv projection
KC = INNER // 128  # 4 contraction chunks for the output projection
NPAIR = HEADS // 2  # 4 head pairs for the packed CT matmuls
REPLICA_GROUPS = [[0, 1], [2, 3], [4, 5], [6, 7]]

_BUILT = None
LAST_RESULT = {}


def build_kernel(debug: bool = False):
    import concourse.bass as bass
    import concourse.mybir as mybir
    import concourse.tile as tile

    BF = mybir.dt.bfloat16
    F32 = mybir.dt.float32
    EXP = mybir.ActivationFunctionType.Exp
    COPY = mybir.ActivationFunctionType.Copy
    X = mybir.AxisListType.X

    nc = bass.Bass(name="linattn")
    xT = nc.declare_dram_parameter("xT", [D, NTOK], BF, isOutput=False)
    wqT = nc.declare_dram_parameter("wqT", [D, INNER], BF, isOutput=False)
    wkvT = nc.declare_dram_parameter("wkvT", [D, 2 * INNER], BF, isOutput=False)
    woutT = nc.declare_dram_parameter("woutT", [INNER, D], BF, isOutput=False)
    bias = nc.declare_dram_parameter("bias", [128, D], F32, isOutput=False)
    y = nc.declare_dram_parameter("y", [NTOK, D], F32, isOutput=True)

    with contextlib.ExitStack() as ctx:
        tc = ctx.enter_context(tile.TileContext(nc))
        cpool = ctx.enter_context(tc.tile_pool(name="const", bufs=1))
        wpool = ctx.enter_context(tc.tile_pool(name="work", bufs=4))
        opool = ctx.enter_context(tc.tile_pool(name="yout", bufs=3))
        dpool = ctx.enter_context(tc.tile_pool(name="dram", bufs=1, space="DRAM"))

        # ---- PE warmup ------------------------------------------------------
        # 6 x 512-col matmuls keep the PE busy (and its p-state ramping)
        # through the ~11us fixed DMA spin-up, so q0 starts near full clock.
        # (The ~11.4us one-time CC-stream init hides behind the combined
        # phase: stage 0 triggers ~60us, its sum isn't needed until ~92us.)
        warm = cpool.tile([128, 512], BF, name="warm")
        nc.vector.memset(warm[:], 0.0)
        ps_warm_cm = tc.tile_pool(name="ps_warm", bufs=1, space="PSUM")
        ps_warm = ps_warm_cm.__enter__()
        warm_ps = ps_warm.tile([128, 512], F32, name="warm_ps")
        for _ in range(14):
            nc.tensor.matmul(
                warm_ps[:], lhsT=warm[:, :128], rhs=warm[:], start=True, stop=True
            )
        ps_warm_cm.__exit__(None, None, None)

        # ---- resident SBUF tensors ----------------------------------------
        wkvT_r = wkvT.rearrange("(ck p) f -> p ck f", p=128)
        xT_r = xT.rearrange("(ck p) n -> p ck n", p=128)
        wqT_r = wqT.rearrange("(ck p) f -> p ck f", p=128)
        wkv_s = cpool.tile([128, CK, 2 * INNER], BF, name="wkv_s")
        x_s = cpool.tile([128, CK, NTOK], BF, name="x_s")
        wq_s = cpool.tile([128, CK, INNER], BF, name="wq_s")
        wout_s = cpool.tile([128, KC, D], BF, name="wout_s")
        bias_s = cpool.tile([128, D], F32, name="bias_s")
        qT_s = cpool.tile([128, KC, NTOK], BF, name="qT_s")
        g_s = cpool.tile([128, KC, D], BF, name="g_s")
        expk_s = cpool.tile([128, NT, INNER], BF, name="expk_s")
        rec_s = cpool.tile([128, NT, HEADS], F32, name="rec_s")

        # ---- input DMA program --------------------------------------------
        # THREE input rings (sync, gpsimd, scalar -- the only engines that
        # can issue DMAs), ~145 GB/s each, in consumption order:
        #   sync:   wq ck-chunks (q0 pace) -> wkv-k -> wout -> bias
        #   gpsimd: x tokens 0:512 ck-chunks (q0 pace) -> strips 1024:1536
        #           -> wkv-v back half
        #   scalar: x tokens 512:1024 ck-chunks -> wkv-v front half
        #           -> strips 1536:2048
        # This cuts the input-arrival critical path from ~40us (2 rings) to
        # ~28us, which directly shortens the laggard core's DMA-paced
        # prologue (exec time tracks the laggard).
        for ck in range(CK):
            nc.sync.dma_start(wq_s[:, ck], wqT_r[:, ck])
            nc.gpsimd.dma_start(x_s[:, ck, :512], xT_r[:, ck, :512])
            nc.scalar.dma_start(x_s[:, ck, 512:1024], xT_r[:, ck, 512:1024])
        nc.sync.dma_start(wkv_s[:, :4, :INNER], wkvT_r[:, :4, :INNER])
        nc.sync.dma_start(wkv_s[:, 4:, :INNER], wkvT_r[:, 4:, :INNER])
        nc.scalar.dma_start(wkv_s[:, :4, INNER:], wkvT_r[:, :4, INNER:])
        # back-half x in per-tile strips so k-tile t waits only on its own
        # 256 KiB (a 1 MiB chunk was measured stalling tiles 8-15 for ~11us)
        for j in range(4):
            tsl = slice(1024 + j * 128, 1024 + (j + 1) * 128)
            nc.gpsimd.dma_start(x_s[:, :, tsl], xT_r[:, :, tsl])
        nc.gpsimd.dma_start(wkv_s[:, 4:, INNER:], wkvT_r[:, 4:, INNER:])
        for j in range(4, 8):
            tsl = slice(1024 + j * 128, 1024 + (j + 1) * 128)
            nc.scalar.dma_start(x_s[:, :, tsl], xT_r[:, :, tsl])
        nc.sync.dma_start(wout_s[:], woutT.rearrange("(kc p) f -> p kc f", p=128))
        nc.sync.dma_start(bias_s[:], bias[:])

        wkv_t = [wkv_s[:, ck] for ck in range(CK)]
        x_t = [x_s[:, ck] for ck in range(CK)]

        # ---- phase B: q projection for token blocks 0,1 -------------------
        # ck-outer / i-inner so each (wq chunk, x chunk) pair is consumed the
        # moment it lands; 4 PSUM banks accumulate the 4 i-chunks.
        ps_qa_cm = tc.tile_pool(name="ps_qa", bufs=5, space="PSUM")
        ps_qa = ps_qa_cm.__enter__()
        for nt2 in range(2):
            tsl = slice(nt2 * 512, (nt2 + 1) * 512)
            qa = [ps_qa.tile([128, 512], F32, name="qa") for i in range(KC)]
            for ck in range(CK):
                for i in range(KC):
                    nc.tensor.matmul(
                        qa[i][:],
                        lhsT=wq_s[:, ck, i * 128 : (i + 1) * 128],
                        rhs=x_t[ck][:, tsl],
                        start=(ck == 0),
                        stop=(ck == CK - 1),
                    )
            for i in range(KC):
                nc.scalar.activation(qT_s[:, i, tsl], qa[i][:], COPY)
        ps_qa_cm.__exit__(None, None, None)

        # ---- phases C/D: k/v projections + packed CT + pair collectives ---
        # Processing order: k-only tiles 0-3 (only the k-half of wkv has
        # landed), combined k+v tiles 4-15, v-only coda tiles 0-3.  The CT
        # stages follow v-processing order [4..15, 0..3], so stage 0 (v of
        # tiles 4-11) triggers its AllGather mid-phase where it hides, and
        # stage 1 closes right before the q remainder.
        # CT[e, d] = sum_n v[n,e]/s[n,h] * exp(k[n,d]); pair-packed: one
        # 128-col matmul covers heads (2p, 2p+1), off-diagonal 64x64 blocks
        # are garbage we ignore.  CT at position j is emitted after the v
        # matmuls of position j+1 so the DVE vsc chain never stalls the PE
        # FIFO.
        ps_k_cm = tc.tile_pool(name="ps_k", bufs=2, space="PSUM")
        ps_k = ps_k_cm.__enter__()
        ps_ct_cm = tc.tile_pool(name="ps_ct", bufs=2, space="PSUM")
        ps_ct = ps_ct_cm.__enter__()
        ps_v_cm = tc.tile_pool(name="ps_v", bufs=2, space="PSUM")
        ps_v = ps_v_cm.__enter__()
        # stage 0 = first 4 v-tiles: it exists to (a) rendezvous the pair
        # early and (b) complete BEFORE stage 1's input is ready, so stage
        # 1's trigger is not serialized behind it on the gpsimd queue
        # (measured cost of a late stage 0: ~20us of blocked stage-1
        # trigger + a ~24us PE gap before G).
        SPLIT = 4
        VORDER = list(range(4, NT)) + list(range(4))
        vsc_t = {}
        ct_ps = [None, None]
        ct_f = [
            cpool.tile([128, KC * DH], F32, name=f"ct_f{i}") for i in range(2)
        ]
        ct_r = cpool.tile([128, KC, DH], F32, name="ct_r")
        ctw = cpool.tile([128, KC, 2, DH], BF, name="ctw")
        nc.vector.memset(ctw[:], 0.0)
        ct_h = [
            cpool.tile([128, 2 * KC * DH], F32, name=f"ct_h{i}") for i in range(2)
        ]
        ct_sum = [
            cpool.tile([128, KC * DH], F32, name=f"ct_sum{i}") for i in range(2)
        ]
        cin = [dpool.tile([128, KC, DH], F32, name=f"cc_in{i}") for i in range(2)]
        cout = [dpool.tile([2, 128, KC, DH], F32, name=f"cc_out{i}") for i in range(2)]

        def k_tile(nt):
            nsl = slice(nt * 128, (nt + 1) * 128)
            k_ps = ps_k.tile([128, INNER], F32, name="k_ps")
            for ck in range(CK):
                nc.tensor.matmul(
                    k_ps[:],
                    lhsT=x_t[ck][:, nsl],
                    rhs=wkv_t[ck][:, :INNER],
                    start=(ck == 0),
                    stop=(ck == CK - 1),
                )
            nc.scalar.activation(expk_s[:, nt], k_ps[:], EXP)
            ssum = wpool.tile([128, HEADS], F32, name="ssum")
            nc.vector.reduce_sum(
                ssum[:], expk_s[:, nt].rearrange("p (h d) -> p h d", d=DH), axis=X
            )
            nc.vector.reciprocal(rec_s[:, nt], ssum[:])

        def v_tile(nt):
            nsl = slice(nt * 128, (nt + 1) * 128)
            v_ps = ps_v.tile([128, INNER], F32, name="v_ps")
            for ck in range(CK):
                nc.tensor.matmul(
                    v_ps[:],
                    lhsT=x_t[ck][:, nsl],
                    rhs=wkv_t[ck][:, INNER:],
                    start=(ck == 0),
                    stop=(ck == CK - 1),
                )
            vsc_t[nt] = wpool.tile([128, INNER], BF, name="vsc")
            nc.vector.tensor_tensor(
                vsc_t[nt][:].rearrange("p (h d) -> p h d", d=DH),
                v_ps[:].rearrange("p (h d) -> p h d", d=DH),
                rec_s[:, nt][:, :, None].to_broadcast([128, HEADS, DH]),
                mybir.AluOpType.mult,
            )

        def ct_mm(j):
            nt = VORDER[j]
            st = 0 if j < SPLIT else 1
            if j == 0 or j == SPLIT:
                ct_ps[st] = ps_ct.tile([128, NPAIR * 128], F32, name="ct_ps")
            for pr in range(NPAIR):
                psl = slice(pr * 128, (pr + 1) * 128)
                # start only on pr==0: start marks the WHOLE bank
                # pending-zero on the addressed partitions (all 128 here),
                # so later pairs' first writes overwrite pending-zero
                # elements; a start on pr>0 would nuke pr<k's stage-first
                # sums.
                nc.tensor.matmul(
                    ct_ps[st][:, psl],
                    lhsT=vsc_t[nt][:, psl],
                    rhs=expk_s[:, nt, psl],
                    start=((j == 0 or j == SPLIT) and pr == 0),
                    stop=(j == SPLIT - 1 or j == NT - 1),
                    skip_group_check=True,
                )

        def ct_stage_out(st):
            # ct_ps layout [128, pr, hf, 64]: diagonal blocks are
            # (part 0:64,  hf=0) = CT of even head 2pr
            # (part 64:128, hf=1) = CT of odd head 2pr+1
            # -> packed ct_f layout: even heads partitions 0:63, odd heads
            # 64:127, columns pair-major (what the collective wants)
            v4 = ct_ps[st][:].rearrange("p (pr hf d) -> p pr hf d", hf=2, d=DH)
            f2 = ct_f[st][:].rearrange("p (pr d) -> p pr d", d=DH)
            nc.scalar.activation(f2[:DH], v4[:DH, :, 0, :], COPY)
            nc.vector.tensor_copy(f2[DH:], v4[DH:, :, 1, :])
            nc.gpsimd.dma_start(
                cin[st].rearrange("p k d -> p (k d)"), ct_f[st][:]
            )
            nc.gpsimd.collective_compute(
                "AllGather",
                mybir.AluOpType.bypass,
                replica_groups=REPLICA_GROUPS,
                ins=[cin[st].opt()],
                outs=[cout[st].opt()],
            )
            # rank-sum on gpsimd: its queue is already blocked on this
            # stage's collective; putting these on the vector queue would
            # stall the v-phase vsc chain behind the collective wait
            nc.gpsimd.dma_start(
                ct_h[st].rearrange("p (r k d) -> p r k d", r=2, d=DH),
                cout[st].rearrange("r p k d -> p r k d"),
            )
            nc.gpsimd.tensor_add(
                ct_sum[st][:], ct_h[st][:, : KC * DH], ct_h[st][:, KC * DH :]
            )

        for nt in range(4):
            k_tile(nt)
        for j in range(12):
            nt = VORDER[j]
            k_tile(nt)
            v_tile(nt)
            if j > 0:
                ct_mm(j - 1)
            if j == SPLIT:
                ct_stage_out(0)
        for j in range(12, NT):
            v_tile(VORDER[j])
            ct_mm(j - 1)
        ps_v_cm.__exit__(None, None, None)

        # ---- phase E: q projection for token blocks 2,3 -------------------
        # first 8 matmuls cover the last vsc DVE chain, then the final CT
        # and the stage-1 collective launch; the rest of q covers the
        # collective + context epilogue.
        ps_qb_cm = tc.tile_pool(name="ps_qb", bufs=3, space="PSUM")
        ps_qb = ps_qb_cm.__enter__()

        def q_block(i, nt2):
            tsl = slice(nt2 * 512, (nt2 + 1) * 512)
            q_ps = ps_qb.tile([128, 512], F32, name="q_ps")
            for ck in range(CK):
                nc.tensor.matmul(
                    q_ps[:],
                    lhsT=wq_s[:, ck, i * 128 : (i + 1) * 128],
                    rhs=x_t[ck][:, tsl],
                    start=(ck == 0),
                    stop=(ck == CK - 1),
            )
            nc.scalar.activation(qT_s[:, i, tsl], q_ps[:], COPY)

        # final CT + stage-1 launch FIRST (before any q): every cycle the
        # trigger moves earlier is a cycle of pair-skew the q remainder can
        # absorb before G needs the summed context
        ct_mm(NT - 1)
        ct_stage_out(1)
        nc.vector.tensor_add(
            ct_r.rearrange("p k d -> p (k d)"), ct_sum[0][:], ct_sum[1][:]
        )
        nc.vector.tensor_copy(ctw[:DH, :, 0, :], ct_r[:DH])
        nc.vector.tensor_copy(ctw[DH:, :, 1, :], ct_r[DH:])
        q_block(0, 2)
        q_block(0, 3)
        for i in range(1, KC):
            q_block(i, 2)
            q_block(i, 3)
        ps_qb_cm.__exit__(None, None, None)
        ps_ct_cm.__exit__(None, None, None)
        ps_k_cm.__exit__(None, None, None)

        # ---- phase F: G = blockdiag(context^T) @ w_out^T ------------------
        ps_g_cm = tc.tile_pool(name="ps_g", bufs=4, space="PSUM")
        ps_g = ps_g_cm.__enter__()
        for pr in range(KC):
            lhs = ctw[:, pr].rearrange("p two d -> p (two d)")
            for half in range(2):
                hsl = slice(half * 512, (half + 1) * 512)
                g_ps = ps_g.tile([128, 512], F32, name="g_ps")
                nc.tensor.matmul(
                    g_ps[:], lhsT=lhs, rhs=wout_s[:, pr, hsl], start=True, stop=True
                )
                # alternate ACT/DVE so g_s drains in ~2.8us and y tile 0
                # is not paced by a serial chain of 8 ACT copies
                if half == 0:
                    nc.scalar.activation(g_s[:, pr, hsl], g_ps[:], COPY)
                else:
                    nc.vector.tensor_copy(g_s[:, pr, hsl], g_ps[:])
        ps_g_cm.__exit__(None, None, None)

        # ---- phase G: y = q @ G + b ---------------------------------------
        ps_y = ctx.enter_context(tc.tile_pool(name="ps_y", bufs=3, space="PSUM"))
        rings = [nc.sync, nc.scalar, nc.gpsimd]
        for nt in range(NT):
            y_ps = ps_y.tile([128, D], F32, name="y_ps")
            for kc in range(KC):
                q_nt = qT_s[:, kc, nt * 128 : (nt + 1) * 128]
                nc.tensor.matmul(
                    y_ps[:, :512],
                    lhsT=q_nt,
                    rhs=g_s[:, kc, :512],
                    start=(kc == 0),
                    stop=(kc == KC - 1),
                )
                nc.tensor.matmul(
                    y_ps[:, 512:],
                    lhsT=q_nt,
                    rhs=g_s[:, kc, 512:],
                    start=(kc == 0),
                    stop=(kc == KC - 1),
                )
            y_t = opool.tile([128, D], F32, name="y_t")
            if nt >= NT - 6:
                # tail tiles: halves drain on rotating ring pairs so the
                # kernel does not end on a few long single-ring transfers
                for half in range(2):
                    hsl = slice(half * 512, (half + 1) * 512)
                    nc.vector.tensor_add(y_t[:, hsl], y_ps[:, hsl], bias_s[:, hsl])
                    eng = rings[(2 * nt + half) % 3]
                    eng.dma_start(y[nt * 128 : (nt + 1) * 128, hsl], y_t[:, hsl])
            else:
                nc.vector.tensor_add(y_t[:], y_ps[:], bias_s[:])
                eng = rings[nt % 3]
                eng.dma_start(y[nt * 128 : (nt + 1) * 128, :], y_t[:])

    return nc


def _prep_inputs(x, w_qkv, w_out, b_out):
    bf16 = ml_dtypes.bfloat16
    x = np.asarray(x, dtype=np.float32)
    w_qkv = np.asarray(w_qkv, dtype=np.float32)
    w_out = np.asarray(w_out, dtype=np.float32)
    b_out = np.asarray(b_out, dtype=np.float32)

    wqT = np.ascontiguousarray(w_qkv[:INNER].T).astype(bf16)  # [D, 512]
    wkvT = np.ascontiguousarray(w_qkv[INNER:].T).astype(bf16)  # [D, 1024]
    woutT = np.ascontiguousarray(w_out.T).astype(bf16)  # [512, D]
    bias = np.ascontiguousarray(np.broadcast_to(b_out[None, :], (128, D))).astype(
        np.float32
    )
    xs = x.reshape(B, 2, NTOK, D)
    in_maps = []
    for c in range(NCORES):
        xT = np.ascontiguousarray(xs[c // 2, c % 2].T).astype(bf16)  # [D, NTOK]
        in_maps.append(
            {"xT": xT, "wqT": wqT, "wkvT": wkvT, "woutT": woutT, "bias": bias}
        )
    return in_maps


def kernel(x, w_qkv, w_out, b_out):
    global _BUILT
    _install_compile_patch()
    if os.environ.get("BASS_TRACE"):
        _install_ntff_hook()
    from concourse.bass_utils import run_bass_kernel_spmd

    if _BUILT is None:
        _BUILT = build_kernel()
    nc = _BUILT
    in_maps = _prep_inputs(x, w_qkv, w_out, b_out)
    res = run_bass_kernel_spmd(nc, in_maps, core_ids=list(range(NCORES)))
    LAST_RESULT["exec_time_ns"] = res.exec_time_ns
    LAST_RESULT["profile_json"] = res.profile_json
    out = np.empty((B, 2, NTOK, D), dtype=np.float32)
    for c in range(NCORES):
        out[c // 2, c % 2] = res.results[c]["y"]
    return out.reshape(B, SEQ, D)


# revision 5
# speedup vs baseline: 1.1008x; 1.1008x over previous
"""LinearAttention kernel for one TRN2 chip (8 NeuronCores), Bass/Tile.

Math (per batch b):
  qkv = x @ w_qkv.T ; q,k,v split, per-head [n, 64]
  k_s = softmax(k, axis=-1)              (over dh, per token/head)
  context_h = k_s^T @ v                  [64, 64]
  out_h = q_h @ context_h ; y = out @ w_out.T + b
Restructured as:
  CT_h = (v/s)-weighted partial:  CT[e,d] = sum_n v[n,e]/s[n,h] * exp(k[n,d])
  G_h  = context_h @ w_out_h^T   -> G [inner=512, 1024] block rows
  y    = q @ G + b               (single K=512 matmul)

Sharding: 8 shards = (batch, half-sequence); each core computes its
2048 tokens end-to-end; only the tiny per-batch context (128 KiB) is
all-reduced between the two cores sharing a batch.

All-bf16 compute. fp8 was MEASURED numerically unsafe everywhere:
weight quantization is systematic across tokens (4e-2), and per-token
activation noise does NOT average out relative to the context, whose
random part also grows as sqrt(N) (1.8e-2 for either x8-kv or fp8-CT,
vs the 2e-2 gate and bf16's 5.4e-3).

Schedule (v6).  Measured invariants that shaped it: DMA rings start
~9-14us after kernel start and sustain ~145 GB/s each (sync + gpsimd;
scalar can also issue DMAs); the PE p-state needs ~3us of continuous
busy to reach 2.4 GHz and idle gaps can re-throttle it (HAM k=4 =
half clock); the FIRST collective pays ~11.5us one-time CC-stream
init; collective_compute BLOCKS the issuing gpsimd queue until the
transfer completes; exec time tracks the LAGGARD core (cross-core DMA
contention skews cores by 5-15us), so collectives must never gate PE
work that could run earlier.
  A. 14 warmup matmuls keep the PE busy/ramping through the DMA
     spin-up window.
  B. q projection for tokens 0-1023, ck-outer / i-inner into 4
     accumulating PSUM banks, so consumption matches DMA arrival
     chunk-for-chunk (ring1 wq, ring2 x) while wkv-k repays the window.
  C. k-only projection tiles 0-3 (needs just the k-half of wkv).
  D. combined k+v tiles 4-15, then v-only coda tiles 0-3.  Per v-tile:
     8 v matmuls then 4 pair-packed CT matmuls (2 heads per 128-col
     matmul; off-diagonal 64x64 blocks computed-and-ignored; diagonals
     land in the packed layout the collective wants; `start` only on
     the first pair -- start marks the whole PSUM bank pending-zero).
     CT for v-position j issues after the v matmuls of position j+1
     (one-tile lag) so the DVE vsc chain never head-of-line blocks the
     PE FIFO.  CT stage 0 = first 4 v-tiles: it rendezvouses the pair
     EARLY and completes before stage 1's input exists, so stage 1's
     trigger is not serialized behind it on the gpsimd queue.
  E. final CT + stage-1 collective launch IMMEDIATELY after the coda,
     then the q projection for tokens 1024-2047 absorbs the partner
     skew + transfer + rank-sum chain.
  F. G = blockdiag(context) @ w_out^T (g_s drains on ACT and DVE in
     parallel), then y = q @ G + b; y drains on three DMA rings with
     the last 6 tiles split in halves across ring pairs.
"""

import contextlib
import ctypes
import os
import sys
import types

import numpy as np
import ml_dtypes

# ---------------------------------------------------------------------------
# Compat shim 1: the walrus in this image supports only ONE semaphore wait
# per instruction; split multi-wait instructions into prefix NoOps.
# ---------------------------------------------------------------------------
_MAX_WAITS = 1


def _legalize_bir(bir_bytes: bytes) -> bytes:
    import orjson

    bir = orjson.loads(bir_bytes)
    changed = False
    for fn in bir.get("functions", []):
        for blk in fn.get("blocks", []):
            new_insts = []
            for ins in blk.get("instructions", []):
                si = ins.get("sync_info") or {}
                waits = si.get("on_wait") or []
                if len(waits) > _MAX_WAITS:
                    changed = True
                    extra, keep = waits[:-_MAX_WAITS], waits[-_MAX_WAITS:]
                    for i in range(0, len(extra), _MAX_WAITS):
                        new_insts.append(
                            {
                                "name": f"{ins['name']}-ws{i}",
                                "opcode": "NoOp",
                                "engine": ins["engine"],
                                "ins": [],
                                "outs": [],
                                "sync_info": {
                                    "on_update": [],
                                    "on_wait": extra[i : i + _MAX_WAITS],
                                },
                            }
                        )
                    si["on_wait"] = keep
                new_insts.append(ins)
            blk["instructions"] = new_insts
    if not changed:
        return bir_bytes
    return orjson.dumps(bir)


_compile_patched = False


def _install_compile_patch():
    global _compile_patched
    if _compile_patched:
        return
    import concourse.bass2jax as bass2jax

    orig = bass2jax.compile_bir_kernel

    def compile_bir_kernel_legalized(bir_json, tmpdir, neff_name="file.neff"):
        return orig(_legalize_bir(bytes(bir_json)), tmpdir, neff_name=neff_name)

    bass2jax.compile_bir_kernel = compile_bir_kernel_legalized
    _compile_patched = True


# ---------------------------------------------------------------------------
# Compat shim 2: NTFF profiling hook (only needed when BASS_TRACE is set).
# ---------------------------------------------------------------------------
def _install_ntff_hook():
    import antenv

    if "antenv.axon_hooks" in sys.modules:
        return
    so_path = "/opt/axon/libaxon_pjrt.so"

    def _mk(so_path):
        try:
            lib = ctypes.CDLL(so_path)
        except OSError:
            return None
        if not hasattr(lib, "axon_start_nrt_profile"):
            return None
        lib.axon_start_nrt_profile.argtypes = [
            ctypes.POINTER(ctypes.c_int64),
            ctypes.c_size_t,
        ]
        lib.axon_start_nrt_profile.restype = ctypes.c_int64
        lib.axon_stop_nrt_profile.argtypes = [ctypes.c_char_p]
        lib.axon_stop_nrt_profile.restype = ctypes.c_int64

        @contextlib.contextmanager
        def _hook(output_dir, device_ids):
            import jax

            jax.devices()
            if device_ids:
                ids = (ctypes.c_int64 * len(device_ids))(*device_ids)
                rc = lib.axon_start_nrt_profile(ids, len(device_ids))
            else:
                rc = lib.axon_start_nrt_profile(None, 0)
            if rc != 0:
                raise RuntimeError(f"axon_start_nrt_profile rc={rc}")
            try:
                yield
            finally:
                n = lib.axon_stop_nrt_profile(str(output_dir).encode())
                if n < 0:
                    raise RuntimeError(f"axon_stop_nrt_profile rc={n}")

        return _hook

    hook = _mk(so_path)
    mod = types.ModuleType("antenv.axon_hooks")
    mod.get_axon_ntff_profile_hook = lambda: hook
    mod.set_axon_ntff_profile_hook = lambda h: None
    sys.modules["antenv.axon_hooks"] = mod
    antenv.axon_hooks = mod


# ---------------------------------------------------------------------------
# Kernel
# ---------------------------------------------------------------------------
B, SEQ, D = 4, 4096, 1024
HEADS, DH = 8, 64
INNER = HEADS * DH  # 512
NCORES = 8
NTOK = B * SEQ // NCORES  # 2048 tokens per core
NT = NTOK // 128  # 16
CK = D // 128  # 8 contraction chunks for the qkv projection
KC = INNER // 128  # 4 contraction chunks for the output projection
NPAIR = HEADS // 2  # 4 head pairs for the packed CT matmuls
REPLICA_GROUPS = [[0, 1], [2, 3], [4, 5], [6, 7]]

_BUILT = None
LAST_RESULT = {}


def build_kernel(debug: bool = False):
    import concourse.bass as bass
    import concourse.mybir as mybir
    import concourse.tile as tile

    BF = mybir.dt.bfloat16
    F32 = mybir.dt.float32
    EXP = mybir.ActivationFunctionType.Exp
    COPY = mybir.ActivationFunctionType.Copy
    X = mybir.AxisListType.X

    nc = bass.Bass(name="linattn")
    xT = nc.declare_dram_parameter("xT", [D, NTOK], BF, isOutput=False)
    wqT = nc.declare_dram_parameter("wqT", [D, INNER], BF, isOutput=False)
    wkvT = nc.declare_dram_parameter("wkvT", [D, 2 * INNER], BF, isOutput=False)
    woutT = nc.declare_dram_parameter("woutT", [INNER, D], BF, isOutput=False)
    bias = nc.declare_dram_parameter("bias", [128, D], F32, isOutput=False)
    y = nc.declare_dram_parameter("y", [NTOK, D], F32, isOutput=True)

    with contextlib.ExitStack() as ctx:
        tc = ctx.enter_context(tile.TileContext(nc))
        cpool = ctx.enter_context(tc.tile_pool(name="const", bufs=1))
        wpool = ctx.enter_context(tc.tile_pool(name="work", bufs=4))
        opool = ctx.enter_context(tc.tile_pool(name="yout", bufs=3))
        dpool = ctx.enter_context(tc.tile_pool(name="dram", bufs=1, space="DRAM"))

        # ---- PE warmup ------------------------------------------------------
        # 6 x 512-col matmuls keep the PE busy (and its p-state ramping)
        # through the ~11us fixed DMA spin-up, so q0 starts near full clock.
        # (The ~11.4us one-time CC-stream init hides behind the combined
        # phase: stage 0 triggers ~60us, its sum isn't needed until ~92us.)
        warm = cpool.tile([128, 512], BF, name="warm")
        nc.vector.memset(warm[:], 0.0)
        ps_warm_cm = tc.tile_pool(name="ps_warm", bufs=1, space="PSUM")
        ps_warm = ps_warm_cm.__enter__()
        warm_ps = ps_warm.tile([128, 512], F32, name="warm_ps")
        for _ in range(14):
            nc.tensor.matmul(
                warm_ps[:], lhsT=warm[:, :128], rhs=warm[:], start=True, stop=True
            )
        ps_warm_cm.__exit__(None, None, None)

        # ---- resident SBUF tensors ----------------------------------------
        wkvT_r = wkvT.rearrange("(ck p) f -> p ck f", p=128)
        xT_r = xT.rearrange("(ck p) n -> p ck n", p=128)
        wqT_r = wqT.rearrange("(ck p) f -> p ck f", p=128)
        wkv_s = cpool.tile([128, CK, 2 * INNER], BF, name="wkv_s")
        x_s = cpool.tile([128, CK, NTOK], BF, name="x_s")
        wq_s = cpool.tile([128, CK, INNER], BF, name="wq_s")
        wout_s = cpool.tile([128, KC, D], BF, name="wout_s")
        bias_s = cpool.tile([128, D], F32, name="bias_s")
        qT_s = cpool.tile([128, KC, NTOK], BF, name="qT_s")
        g_s = cpool.tile([128, KC, D], BF, name="g_s")
        expk_s = cpool.tile([128, NT, INNER], BF, name="expk_s")
        rec_s = cpool.tile([128, NT, HEADS], F32, name="rec_s")

        # ---- input DMA program --------------------------------------------
        # ring1 (sync): wq ck-chunks in q0 consumption order, then wkv k-half,
        # wkv v-half front, wout, bias.  ring2 (gpsimd): x token-blocks 0:512
        # and 512:1024 ck-chunked (q0 order), wkv v-half back, then the back
        # half of x token-major.  Each ring sustains ~145 GB/s; q0 consumes
        # both rings chunk-for-chunk while wkv-k repays the window.
        # (A 3rd input ring on the scalar engine was measured SLOWER: it
        # couples the 8 cores' HBM traffic and inflates the laggard.)
        for ck in range(CK):
            nc.sync.dma_start(wq_s[:, ck], wqT_r[:, ck])
            nc.gpsimd.dma_start(x_s[:, ck, :512], xT_r[:, ck, :512])
        for ck in range(CK):
            nc.gpsimd.dma_start(x_s[:, ck, 512:1024], xT_r[:, ck, 512:1024])
        nc.sync.dma_start(wkv_s[:, :4, :INNER], wkvT_r[:, :4, :INNER])
        nc.sync.dma_start(wkv_s[:, 4:, :INNER], wkvT_r[:, 4:, :INNER])
        # wkv-v split across both rings so the combined k+v phase is never
        # gated on a single ring finishing it
        nc.sync.dma_start(wkv_s[:, :4, INNER:], wkvT_r[:, :4, INNER:])
        nc.gpsimd.dma_start(wkv_s[:, 4:, INNER:], wkvT_r[:, 4:, INNER:])
        # back-half x in per-tile strips so k-tile t waits only on its own
        # 256 KiB (a 1 MiB chunk was measured stalling tiles 8-15 for ~11us)
        for j in range(8):
            tsl = slice(1024 + j * 128, 1024 + (j + 1) * 128)
            nc.gpsimd.dma_start(x_s[:, :, tsl], xT_r[:, :, tsl])
        nc.sync.dma_start(wout_s[:], woutT.rearrange("(kc p) f -> p kc f", p=128))
        nc.sync.dma_start(bias_s[:], bias[:])

        wkv_t = [wkv_s[:, ck] for ck in range(CK)]
        x_t = [x_s[:, ck] for ck in range(CK)]

        # ---- phase B: q projection for token blocks 0,1 -------------------
        # ck-outer / i-inner so each (wq chunk, x chunk) pair is consumed the
        # moment it lands; 4 PSUM banks accumulate the 4 i-chunks.
        ps_qa_cm = tc.tile_pool(name="ps_qa", bufs=5, space="PSUM")
        ps_qa = ps_qa_cm.__enter__()
        for nt2 in range(2):
            tsl = slice(nt2 * 512, (nt2 + 1) * 512)
            qa = [ps_qa.tile([128, 512], F32, name="qa") for i in range(KC)]
            for ck in range(CK):
                for i in range(KC):
                    nc.tensor.matmul(
                        qa[i][:],
                        lhsT=wq_s[:, ck, i * 128 : (i + 1) * 128],
                        rhs=x_t[ck][:, tsl],
                        start=(ck == 0),
                        stop=(ck == CK - 1),
                    )
            for i in range(KC):
                nc.scalar.activation(qT_s[:, i, tsl], qa[i][:], COPY)
        ps_qa_cm.__exit__(None, None, None)

        # ---- phases C/D: k/v projections + packed CT + pair collectives ---
        # Processing order: k-only tiles 0-3 (only the k-half of wkv has
        # landed), combined k+v tiles 4-15, v-only coda tiles 0-3.  The CT
        # stages follow v-processing order [4..15, 0..3], so stage 0 (v of
        # tiles 4-11) triggers its AllGather mid-phase where it hides, and
        # stage 1 closes right before the q remainder.
        # CT[e, d] = sum_n v[n,e]/s[n,h] * exp(k[n,d]); pair-packed: one
        # 128-col matmul covers heads (2p, 2p+1), off-diagonal 64x64 blocks
        # are garbage we ignore.  CT at position j is emitted after the v
        # matmuls of position j+1 so the DVE vsc chain never stalls the PE
        # FIFO.
        ps_k_cm = tc.tile_pool(name="ps_k", bufs=2, space="PSUM")
        ps_k = ps_k_cm.__enter__()
        ps_ct_cm = tc.tile_pool(name="ps_ct", bufs=2, space="PSUM")
        ps_ct = ps_ct_cm.__enter__()
        ps_v_cm = tc.tile_pool(name="ps_v", bufs=2, space="PSUM")
        ps_v = ps_v_cm.__enter__()
        # stage 0 = first 4 v-tiles: it exists to (a) rendezvous the pair
        # early and (b) complete BEFORE stage 1's input is ready, so stage
        # 1's trigger is not serialized behind it on the gpsimd queue
        # (measured cost of a late stage 0: ~20us of blocked stage-1
        # trigger + a ~24us PE gap before G).
        SPLIT = 4
        VORDER = list(range(4, NT)) + list(range(4))
        vsc_t = {}
        ct_ps = [None, None]
        ct_f = [
            cpool.tile([128, KC * DH], F32, name=f"ct_f{i}") for i in range(2)
        ]
        ct_r = cpool.tile([128, KC, DH], F32, name="ct_r")
        ctw = cpool.tile([128, KC, 2, DH], BF, name="ctw")
        nc.vector.memset(ctw[:], 0.0)
        ct_h = [
            cpool.tile([128, 2 * KC * DH], F32, name=f"ct_h{i}") for i in range(2)
        ]
        ct_sum = [
            cpool.tile([128, KC * DH], F32, name=f"ct_sum{i}") for i in range(2)
        ]
        cin = [dpool.tile([128, KC, DH], F32, name=f"cc_in{i}") for i in range(2)]
        cout = [dpool.tile([2, 128, KC, DH], F32, name=f"cc_out{i}") for i in range(2)]

        def k_tile(nt):
            nsl = slice(nt * 128, (nt + 1) * 128)
            k_ps = ps_k.tile([128, INNER], F32, name="k_ps")
            for ck in range(CK):
                nc.tensor.matmul(
                    k_ps[:],
                    lhsT=x_t[ck][:, nsl],
                    rhs=wkv_t[ck][:, :INNER],
                    start=(ck == 0),
                    stop=(ck == CK - 1),
                )
            nc.scalar.activation(expk_s[:, nt], k_ps[:], EXP)
            ssum = wpool.tile([128, HEADS], F32, name="ssum")
            nc.vector.reduce_sum(
                ssum[:], expk_s[:, nt].rearrange("p (h d) -> p h d", d=DH), axis=X
            )
            nc.vector.reciprocal(rec_s[:, nt], ssum[:])

        def v_tile(nt):
            nsl = slice(nt * 128, (nt + 1) * 128)
            v_ps = ps_v.tile([128, INNER], F32, name="v_ps")
            for ck in range(CK):
                nc.tensor.matmul(
                    v_ps[:],
                    lhsT=x_t[ck][:, nsl],
                    rhs=wkv_t[ck][:, INNER:],
                    start=(ck == 0),
                    stop=(ck == CK - 1),
                )
            vsc_t[nt] = wpool.tile([128, INNER], BF, name="vsc")
            nc.vector.tensor_tensor(
                vsc_t[nt][:].rearrange("p (h d) -> p h d", d=DH),
                v_ps[:].rearrange("p (h d) -> p h d", d=DH),
                rec_s[:, nt][:, :, None].to_broadcast([128, HEADS, DH]),
                mybir.AluOpType.mult,
            )

        def ct_mm(j):
            nt = VORDER[j]
            st = 0 if j < SPLIT else 1
            if j == 0 or j == SPLIT:
                ct_ps[st] = ps_ct.tile([128, NPAIR * 128], F32, name="ct_ps")
            for pr in range(NPAIR):
                psl = slice(pr * 128, (pr + 1) * 128)
                # start only on pr==0: start marks the WHOLE bank
                # pending-zero on the addressed partitions (all 128 here),
                # so later pairs' first writes overwrite pending-zero
                # elements; a start on pr>0 would nuke pr<k's stage-first
                # sums.
                nc.tensor.matmul(
                    ct_ps[st][:, psl],
                    lhsT=vsc_t[nt][:, psl],
                    rhs=expk_s[:, nt, psl],
                    start=((j == 0 or j == SPLIT) and pr == 0),
                    stop=(j == SPLIT - 1 or j == NT - 1),
                    skip_group_check=True,
                )

        def ct_stage_out(st):
            # ct_ps layout [128, pr, hf, 64]: diagonal blocks are
            # (part 0:64,  hf=0) = CT of even head 2pr
            # (part 64:128, hf=1) = CT of odd head 2pr+1
            # -> packed ct_f layout: even heads partitions 0:63, odd heads
            # 64:127, columns pair-major (what the collective wants)
            v4 = ct_ps[st][:].rearrange("p (pr hf d) -> p pr hf d", hf=2, d=DH)
            f2 = ct_f[st][:].rearrange("p (pr d) -> p pr d", d=DH)
            nc.scalar.activation(f2[:DH], v4[:DH, :, 0, :], COPY)
            nc.vector.tensor_copy(f2[DH:], v4[DH:, :, 1, :])
            nc.gpsimd.dma_start(
                cin[st].rearrange("p k d -> p (k d)"), ct_f[st][:]
            )
            nc.gpsimd.collective_compute(
                "AllGather",
                mybir.AluOpType.bypass,
                replica_groups=REPLICA_GROUPS,
                ins=[cin[st].opt()],
                outs=[cout[st].opt()],
            )
            # rank-sum on gpsimd: its queue is already blocked on this
            # stage's collective; putting these on the vector queue would
            # stall the v-phase vsc chain behind the collective wait
            nc.gpsimd.dma_start(
                ct_h[st].rearrange("p (r k d) -> p r k d", r=2, d=DH),
                cout[st].rearrange("r p k d -> p r k d"),
            )
            nc.gpsimd.tensor_add(
                ct_sum[st][:], ct_h[st][:, : KC * DH], ct_h[st][:, KC * DH :]
            )

        for nt in range(4):
            k_tile(nt)
        for j in range(12):
            nt = VORDER[j]
            k_tile(nt)
            v_tile(nt)
            if j > 0:
                ct_mm(j - 1)
            if j == SPLIT:
                ct_stage_out(0)
        for j in range(12, NT):
            v_tile(VORDER[j])
            ct_mm(j - 1)
        ps_v_cm.__exit__(None, None, None)

        # ---- phase E: q projection for token blocks 2,3 -------------------
        # first 8 matmuls cover the last vsc DVE chain, then the final CT
        # and the stage-1 collective launch; the rest of q covers the
        # collective + context epilogue.
        ps_qb_cm = tc.tile_pool(name="ps_qb", bufs=3, space="PSUM")
        ps_qb = ps_qb_cm.__enter__()

        def q_block(i, nt2):
            tsl = slice(nt2 * 512, (nt2 + 1) * 512)
            q_ps = ps_qb.tile([128, 512], F32, name="q_ps")
            for ck in range(CK):
                nc.tensor.matmul(
                    q_ps[:],
                    lhsT=wq_s[:, ck, i * 128 : (i + 1) * 128],
                    rhs=x_t[ck][:, tsl],
                    start=(ck == 0),
                    stop=(ck == CK - 1),
            )
            nc.scalar.activation(qT_s[:, i, tsl], q_ps[:], COPY)

        # final CT + stage-1 launch FIRST (before any q): every cycle the
        # trigger moves earlier is a cycle of pair-skew the q remainder can
        # absorb before G needs the summed context
        ct_mm(NT - 1)
        ct_stage_out(1)
        nc.vector.tensor_add(
            ct_r.rearrange("p k d -> p (k d)"), ct_sum[0][:], ct_sum[1][:]
        )
        nc.vector.tensor_copy(ctw[:DH, :, 0, :], ct_r[:DH])
        nc.vector.tensor_copy(ctw[DH:, :, 1, :], ct_r[DH:])
        q_block(0, 2)
        q_block(0, 3)
        for i in range(1, KC):
            q_block(i, 2)
            q_block(i, 3)
        ps_qb_cm.__exit__(None, None, None)
        ps_ct_cm.__exit__(None, None, None)
        ps_k_cm.__exit__(None, None, None)

        # ---- phase F: G = blockdiag(context^T) @ w_out^T ------------------
        ps_g_cm = tc.tile_pool(name="ps_g", bufs=4, space="PSUM")
        ps_g = ps_g_cm.__enter__()
        for pr in range(KC):
            lhs = ctw[:, pr].rearrange("p two d -> p (two d)")
            for half in range(2):
                hsl = slice(half * 512, (half + 1) * 512)
                g_ps = ps_g.tile([128, 512], F32, name="g_ps")
                nc.tensor.matmul(
                    g_ps[:], lhsT=lhs, rhs=wout_s[:, pr, hsl], start=True, stop=True
                )
                # alternate ACT/DVE so g_s drains in ~2.8us and y tile 0
                # is not paced by a serial chain of 8 ACT copies
                if half == 0:
                    nc.scalar.activation(g_s[:, pr, hsl], g_ps[:], COPY)
                else:
                    nc.vector.tensor_copy(g_s[:, pr, hsl], g_ps[:])
        ps_g_cm.__exit__(None, None, None)

        # ---- phase G: y = q @ G + b ---------------------------------------
        ps_y = ctx.enter_context(tc.tile_pool(name="ps_y", bufs=3, space="PSUM"))
        rings = [nc.sync, nc.scalar, nc.gpsimd]
        for nt in range(NT):
            y_ps = ps_y.tile([128, D], F32, name="y_ps")
            for kc in range(KC):
                q_nt = qT_s[:, kc, nt * 128 : (nt + 1) * 128]
                nc.tensor.matmul(
                    y_ps[:, :512],
                    lhsT=q_nt,
                    rhs=g_s[:, kc, :512],
                    start=(kc == 0),
                    stop=(kc == KC - 1),
                )
                nc.tensor.matmul(
                    y_ps[:, 512:],
                    lhsT=q_nt,
                    rhs=g_s[:, kc, 512:],
                    start=(kc == 0),
                    stop=(kc == KC - 1),
                )
            y_t = opool.tile([128, D], F32, name="y_t")
            if nt >= NT - 6:
                # tail tiles: halves drain on rotating ring pairs so the
                # kernel does not end on a few long single-ring transfers
                for half in range(2):
                    hsl = slice(half * 512, (half + 1) * 512)
                    nc.vector.tensor_add(y_t[:, hsl], y_ps[:, hsl], bias_s[:, hsl])
                    eng = rings[(2 * nt + half) % 3]
                    eng.dma_start(y[nt * 128 : (nt + 1) * 128, hsl], y_t[:, hsl])
            else:
                nc.vector.tensor_add(y_t[:], y_ps[:], bias_s[:])
                eng = rings[nt % 3]
                eng.dma_start(y[nt * 128 : (nt + 1) * 128, :], y_t[:])

    return nc


def _prep_inputs(x, w_qkv, w_out, b_out):
    bf16 = ml_dtypes.bfloat16
    x = np.asarray(x, dtype=np.float32)
    w_qkv = np.asarray(w_qkv, dtype=np.float32)
    w_out = np.asarray(w_out, dtype=np.float32)
    b_out = np.asarray(b_out, dtype=np.float32)

    wqT = np.ascontiguousarray(w_qkv[:INNER].T).astype(bf16)  # [D, 512]
    wkvT = np.ascontiguousarray(w_qkv[INNER:].T).astype(bf16)  # [D, 1024]
    woutT = np.ascontiguousarray(w_out.T).astype(bf16)  # [512, D]
    bias = np.ascontiguousarray(np.broadcast_to(b_out[None, :], (128, D))).astype(
        np.float32
    )
    xs = x.reshape(B, 2, NTOK, D)
    in_maps = []
    for c in range(NCORES):
        xT = np.ascontiguousarray(xs[c // 2, c % 2].T).astype(bf16)  # [D, NTOK]
        in_maps.append(
            {"xT": xT, "wqT": wqT, "wkvT": wkvT, "woutT": woutT, "bias": bias}
        )
    return in_maps


def kernel(x, w_qkv, w_out, b_out):
    global _BUILT
    _install_compile_patch()
    if os.environ.get("BASS_TRACE"):
        _install_ntff_hook()
    from concourse.bass_utils import run_bass_kernel_spmd

    if _BUILT is None:
        _BUILT = build_kernel()
    nc = _BUILT
    in_maps = _prep_inputs(x, w_qkv, w_out, b_out)
    res = run_bass_kernel_spmd(nc, in_maps, core_ids=list(range(NCORES)))
    LAST_RESULT["exec_time_ns"] = res.exec_time_ns
    LAST_RESULT["profile_json"] = res.profile_json
    out = np.empty((B, 2, NTOK, D), dtype=np.float32)
    for c in range(NCORES):
        out[c // 2, c % 2] = res.results[c]["y"]
    return out.reshape(B, SEQ, D)
